# revision 52
# baseline (speedup 1.0000x reference)
"""Trainium2 Bass kernel for nn_CrossAttention (linear attention, elu+1 feature map).

Math (per batch element n of B=4, sequence L = V*HW = 20480, C=256, H=8 heads, d=32):
    qkv = xb @ W_qkv ; q,k,v splits
    phi(t) = elu(t)+1  (exactly min(relu(t)+1, exp(t)))
    kv[h,m,d] = sum_l phi(k)[l,h,d] * v[l,h,m]
    z[l,h]   = 1 / (phi(q)[l,h,:] . sum_l phi(k)[l,h,:] + eps)
    y[l,h,m] = phi(q)[l,h,:] . kv[h,:,m] * z[l,h]
    out      = y @ W_proj + b_proj

Key algebraic restructure vs the v-materializing formulation:
    kv = phi(k)^T @ (x @ W_v) = (phi(k)^T @ [x | 1]) @ W_v = S[:, :C] @ W_v
so phase 1 only projects k and accumulates S = phik^T [x|1] (the ones column
gives ksum = S[:, C]).  S is AllReduced across the core pair (both L-halves),
then each core builds
    kv^T = W_v^T @ S^T        (S^T via 4 PE transposes)
    M    = blockdiag(kv^T)-masked @ W_proj      ([256, 256] dense)
and the output is a single GEMM  out = (phi(q)/den) @ M  -- the separate
y = phiq @ kvblk matmul is folded away.

Sharding: 8 cores = 4 batches x 2 L-halves (LH=10240 rows each); only S
([2,128,258] f32 = 264KB) crosses cores, AllReduced over core pairs.

Scheduling notes (from timeline-sim analysis):
 - every dma_start costs ~625ns on the issuing sequencer AND the shared
   HWDGE, so transfers are batched into 1024-row units;
 - engines are in-order, so phase 1 lags the S matmuls (SLAG pairs) behind
   k-proj/phi, and phase 2 splits front (qt+phi) / mid (dn+divide) / back
   (out+copy+dma) with lags, prefilling fronts to hide the AllReduce;
 - all post-collective build ops (ksx, S^T evict, kv^T mask, M evict) sit on
   Pool, whose per-tile phase-2 op (out-copy) is collective-gated anyway, so
   the DVE/Act prefill streams never block on the collective.
"""

import sys
import numpy as np

if "/opt/trn_rl_repo" not in sys.path:
    sys.path.insert(0, "/opt/trn_rl_repo")

# ---------------- problem constants (hardcoded per contest rules) -----------
BV, HW, C = 20, 4096, 256
NVIEW = 5
B = BV // NVIEW          # 4
H = 8
D = C // H               # 32
L = NVIEW * HW           # 20480
N_CORES = 8
LH = L // 2              # 10240 rows per core
EPS = 1e-6               # folded away: den >> 1e-6 always (phi>0, ksum~2e4)
PREFILL = 5              # phase-2 front pairs issued before the build/mids
SLAG = 2                 # phase-1 S-matmul lag (pairs) behind k-proj/phi

_NC_CACHE = {}


def _build_nc(lh=LH, collective=True, split_waits=True, pool_rq=False,
              xt_f32r=False, copy_act_all=True, front_actrelu=True,
              p1_actrelu=False):
    """Build the Bass program (SPMD, one core's share: x-half -> [lh, C])."""
    import concourse.bass as bass
    import concourse.mybir as mybir
    import concourse.tile as tile
    from contextlib import ExitStack

    f32 = mybir.dt.float32
    f32r = mybir.dt.float32r
    bf16 = mybir.dt.bfloat16
    AF = mybir.ActivationFunctionType
    OP = mybir.AluOpType
    PSUM = bass.MemorySpace.PSUM
    DRAM = bass.MemorySpace.DRAM

    assert lh % 1024 == 0
    NP = lh // 512           # pairs / xst tiles (20): 512 l-rows each

    nc = bass.Bass("TRN2", target_bir_lowering=False, debug=False,
                   num_devices=N_CORES)

    xdt = f32r if xt_f32r else bf16
    xT = nc.dram_tensor("xT", [C, lh], xdt, kind="ExternalInput")
    xn = nc.dram_tensor("xn", [lh, 258], bf16, kind="ExternalInput")
    wqkv = nc.dram_tensor("w_qkv", [C, 3 * C], xdt, kind="ExternalInput")
    wproj = nc.dram_tensor("w_proj", [C, C], bf16, kind="ExternalInput")
    eyeid = nc.dram_tensor("eyeid", [128, 256], f32, kind="ExternalInput")
    out = nc.dram_tensor("out", [lh, C], f32, kind="ExternalOutput")

    # xT tiles: [t, p, hb, l] with source row hb*128+p, col t*512+l
    xT_r = xT[:].rearrange("(hb p) (t l) -> t p hb l", p=128, l=512)
    # xn groups of 4 l-tiles: [i, p, j, f] with source row i*512 + j*128 + p
    xn_r = xn[:].rearrange("(i j p) f -> i p j f", j=4, p=128)
    # out pairs: rows u*512 + j*128 + p
    out_r = out[:].rearrange("(u j p) f -> u p j f", j=4, p=128)

    with tile.TileContext(nc) as tc, ExitStack() as ctx:
        const = ctx.enter_context(tc.tile_pool(name="const", bufs=1))
        stash = ctx.enter_context(tc.tile_pool(name="stash", bufs=1))
        ekp = ctx.enter_context(tc.tile_pool(name="ekp", bufs=3))
        xnp = ctx.enter_context(tc.tile_pool(name="xnp", bufs=SLAG + 3))
        phkp = ctx.enter_context(tc.tile_pool(name="phkp", bufs=SLAG + 2))
        dram = ctx.enter_context(tc.tile_pool(name="dram", bufs=1, space=DRAM))

        # ---- constants -----------------------------------------------------
        w_sb = [const.tile([128, 3 * C], xdt, tag=f"w{h}", name=f"w{h}") for h in range(2)]
        for h in range(2):
            nc.sync.dma_start(w_sb[h][:], wqkv[128 * h:128 * (h + 1), :])
        # post-collective constants declared here, DMA'd mid-phase-1 (below)
        wp_sb = [const.tile([128, C], bf16, tag=f"wp{m}", name=f"wp{m}") for m in range(2)]
        eyeid_sb = const.tile([128, 256], f32, tag="eyeid")
        eye_sb = eyeid_sb[:, 0:128]
        id_sb = eyeid_sb[:, 128:256]
        # masked kv^T blocks (written post-collective; zeroed once up front)
        kvTblk = [const.tile([128, 128], bf16, tag=f"kvTblk{m}", name=f"kvTblk{m}")
                  for m in range(2)]

        # ---- x^T stash (lhsT of phase-1 k-proj; rhs of phase-2 qt) ---------
        xst = [stash.tile([128, 2, 512], xdt, tag=f"x{t}", name=f"x{t}")
               for t in range(NP)]

        # ---- phase 1: k projection + phi(k) + S accumulation ---------------
        with tc.tile_pool(name="ps_s", bufs=1, space=PSUM) as ps_s:
            sps = [ps_s.tile([128, 258], f32, tag=f"s{m}", name=f"sps{m}") for m in range(2)]
            with tc.tile_pool(name="ps_k", bufs=3, space=PSUM) as ps_k:
                phik_xn = {}

                def p1_front(i):
                    nc.sync.dma_start(xst[i][:], xT_r[i])
                    xn_sb = xnp.tile([128, 4, 258], bf16, tag="xn")
                    nc.sync.dma_start(xn_sb[:], xn_r[i])

                    k_ps = ps_k.tile([128, 4, 256], f32, tag="k")
                    for j in range(4):
                        for h in range(2):
                            nc.tensor.matmul(
                                k_ps[:, j, :],
                                xst[i][:, h, 128 * j:128 * (j + 1)],
                                w_sb[h][:, C:2 * C],
                                start=(h == 0), stop=(h == 1))
                    e_k = ekp.tile([128, 4, 256], bf16, tag="e_k")
                    r_k = ekp.tile([128, 4, 256], bf16, tag="r_k")
                    phik = phkp.tile([128, 4, 256], bf16, tag="phik")
                    nc.scalar.activation(e_k[:], k_ps[:], AF.Exp)
                    if p1_actrelu:
                        nc.scalar.activation(r_k[:], k_ps[:], AF.Relu)
                        nc.vector.scalar_tensor_tensor(
                            phik[:], r_k[:], 1.0, e_k[:], op0=OP.add, op1=OP.min)
                    else:
                        nc.vector.tensor_scalar(r_k[:], k_ps[:], 0.0, 1.0,
                                                op0=OP.max, op1=OP.add)
                        nc.vector.tensor_tensor(phik[:], r_k[:], e_k[:], op=OP.min)
                    phik_xn[i] = (phik, xn_sb)

                def p1_back(i):
                    phik, xn_sb = phik_xn.pop(i)
                    for j in range(4):
                        for m in range(2):
                            nc.tensor.matmul(
                                sps[m][:, :],
                                phik[:, j, 128 * m:128 * (m + 1)],
                                xn_sb[:, j, :],
                                start=(i == 0 and j == 0),
                                stop=(i == NP - 1 and j == 3),
                                skip_group_check=True)

                for i in range(NP):
                    p1_front(i)
                    # post-collective consts ride along mid-phase (DMA slack)
                    if i == 1:
                        for m in range(2):
                            nc.sync.dma_start(wp_sb[m][:],
                                              wproj[128 * m:128 * (m + 1), :])
                            nc.gpsimd.memset(kvTblk[m][:], 0.0)
                        nc.sync.dma_start(eyeid_sb[:], eyeid[:, :])
                    if i >= SLAG:
                        p1_back(i - SLAG)
                for i in range(NP - SLAG, NP):
                    p1_back(i)

            # ---- evict S partials ------------------------------------------
            sev = const.tile([128, 2, 258], f32, tag="sev", name="sev")
            for m in range(2):
                nc.vector.tensor_copy(sev[:, m, :], sps[m][:])

        # ---- transpose S pre-collective (linear, so AllReduce(S^T)=Sum^T) --
        # st_pack[:, h, 0:256] = S^T c-half h (dfeat cols); ksum rides along
        # in cols 256:258 of slot m (partition there = dfeat index of half m).
        st_pack = const.tile([128, 2, 258], f32, tag="st_pack", name="st_pack")
        with tc.tile_pool(name="ps_st", bufs=1, space=PSUM) as ps_st:
            st_ps = ps_st.tile([128, 2, 256], f32, tag="st", name="st")
            for h in range(2):
                for m in range(2):
                    nc.tensor.transpose(
                        st_ps[:, h, 128 * m:128 * (m + 1)],
                        sev[:, m, 128 * h:128 * (h + 1)], id_sb)
            for h in range(2):
                nc.vector.tensor_copy(st_pack[:, h, 0:256], st_ps[:, h, :])
            for m in range(2):
                nc.vector.tensor_copy(st_pack[:, m, 256:258],
                                      sev[:, m, 256:258])

        s_in = dram.tile([2, 128, 258], f32, tag="s_in")
        s_out = dram.tile([2, 128, 258], f32, tag="s_out")
        nc.sync.dma_start(s_in[:].rearrange("m p f -> p m f"), st_pack[:])
        if collective:
            nc.gpsimd.collective_compute(
                "AllReduce", mybir.AluOpType.add,
                replica_groups=[[2 * p, 2 * p + 1] for p in range(N_CORES // 2)],
                ins=[s_in[:].opt()],
                outs=[s_out[:].opt()])
        else:  # single-core timeline-sim variant (keep it on the Pool queue)
            nc.gpsimd.dma_start(s_out[:], s_in[:])
        kvr = const.tile([128, 2, 258], f32, tag="kvr")
        nc.sync.dma_start(kvr[:], s_out[:].rearrange("m p f -> p m f"))

        # ---- phase 2: qt, phi(q), den, out = (phiq/den) @ M ----------------
        with tc.tile_pool(name="ps_qt", bufs=2, space=PSUM) as ps_qt, \
             tc.tile_pool(name="phq", bufs=(13 if xt_f32r else NP)) as phq, \
             tc.tile_pool(name="sb2", bufs=3) as sb2:

            phiq_tiles = {}

            def issue_front(u):
                qt_ps = ps_qt.tile([128, 2, 512], f32, tag="qt")
                for m in range(2):
                    for h in range(2):
                        nc.tensor.matmul(
                            qt_ps[:, m, :],
                            w_sb[h][:, 128 * m:128 * (m + 1)],
                            xst[u][:, h, :],
                            start=(h == 0), stop=(h == 1))
                # Act evicts q to SBUF + Exp; Pool (idle otherwise) does
                # relu+1 from SBUF; DVE min runs at 2x (all-bf16 SBUF)
                e_q = sb2.tile([128, 2, 512], bf16, tag="e_q")
                r_q = sb2.tile([128, 2, 512], bf16, tag="r_q")
                phiq = phq.tile([128, 2, 512], bf16, tag="phiq")
                nc.scalar.activation(e_q[:], qt_ps[:], AF.Exp)
                if front_actrelu:
                    nc.scalar.activation(r_q[:], qt_ps[:], AF.Relu)
                    nc.vector.scalar_tensor_tensor(
                        phiq[:], r_q[:], 1.0, e_q[:], op0=OP.add, op1=OP.min)
                else:
                    q_sb = sb2.tile([128, 2, 512], bf16, tag="q_sb")
                    nc.scalar.activation(q_sb[:], qt_ps[:], AF.Copy)
                    nc.vector.tensor_scalar(r_q[:], q_sb[:], 0.0, 1.0,
                                            op0=OP.max, op1=OP.add)
                    nc.vector.tensor_tensor(phiq[:], r_q[:], e_q[:], op=OP.min)
                phiq_tiles[u] = phiq

            # ---- post-collective build (issued at u == PREFILL, inside the
            # loop, so the PREFILL fronts shield Act/DVE/PE streams from the
            # collective wait).  Pool ops (ksx, st_bf) issue up front -- Pool
            # is otherwise idle in phase 2.  The PE build matmuls borrow a
            # rotation slot of the qt pool (no spare PSUM banks).
            ksx = [const.tile([128, 128], bf16, tag=f"ksx{m}", name=f"ksx{m}")
                   for m in range(2)]
            for m in range(2):
                nc.vector.tensor_scalar(
                    ksx[m][:], eye_sb, kvr[:, m, 256:257], None, op0=OP.mult)
            st_bf = const.tile([128, 2, 256], xdt, tag="st_bf", name="st_bf")
            nc.vector.tensor_copy(st_bf[:], kvr[:, :, 0:256])
            m_sb = [const.tile([128, 256], bf16, tag=f"msb{m}", name=f"msb{m}")
                    for m in range(2)]

            def issue_build(ps_bld):
                bld = ps_bld.tile([128, 2, 512], f32, tag="bld", name="bld")
                kvt_ps = bld[:, :, 0:256]
                for mt in range(2):
                    for h in range(2):
                        nc.tensor.matmul(
                            kvt_ps[:, mt, :],
                            w_sb[h][:, 2 * C + 128 * mt:2 * C + 128 * (mt + 1)],
                            st_bf[:, h, :],
                            start=(h == 0), stop=(h == 1))
                for m in range(2):
                    for a in range(4):
                        nc.vector.tensor_copy(
                            kvTblk[m][32 * a:32 * (a + 1), 32 * a:32 * (a + 1)],
                            kvt_ps[32 * a:32 * (a + 1), m,
                                   128 * m + 32 * a:128 * m + 32 * (a + 1)])
                m_ps = bld[:, :, 256:512]
                for m in range(2):
                    nc.tensor.matmul(m_ps[:, m, :], kvTblk[m][:], wp_sb[m][:],
                                     start=True, stop=True)
                    nc.vector.tensor_copy(m_sb[m][:], m_ps[:, m, :])

            # prefill + build in a closed PSUM scope, then the steady pools
            # reuse those banks
            for u in range(PREFILL):
                issue_front(u)
            with tc.tile_pool(name="ps_bld", bufs=1, space=PSUM) as ps_bld:
                issue_build(ps_bld)

            # ---- steady pipeline -------------------------------------------
            with tc.tile_pool(name="ps_dn", bufs=1, space=PSUM) as ps_dn, \
                 tc.tile_pool(name="ps_out", bufs=1, space=PSUM) as ps_out, \
                 tc.tile_pool(name="ysc", bufs=4) as ysc, \
                 tc.tile_pool(name="zxp", bufs=3) as zxp, \
                 tc.tile_pool(name="sb2c", bufs=2) as sb2c:

                y_tiles = {}

                def issue_mid(u):
                    phiq = phiq_tiles.pop(u)
                    dn_ps = ps_dn.tile([128, 2, 512], f32, tag="dn")
                    for m in range(2):
                        nc.tensor.matmul(dn_ps[:, m, :], ksx[m][:],
                                         phiq[:, m, :], start=True, stop=True)
                    zex = zxp.tile([128, 2, 512], bf16, tag="zex")
                    with nc.allow_low_precision(
                            reason="z in bf16: ~4e-3 rel, fine for 2e-2 gate"):
                        nc.vector.reciprocal(zex[:].opt(), dn_ps[:].opt())
                    y_sc = ysc.tile([128, 2, 512], bf16, tag="y_sc")
                    nc.vector.tensor_tensor(y_sc[:], phiq[:], zex[:], op=OP.mult)
                    y_tiles[u] = y_sc

                def issue_back(u):
                    y_sc = y_tiles.pop(u)
                    out_sb = sb2c.tile([128, 4, 256], f32, tag="out_sb")
                    out_ps = ps_out.tile([128, 4, 256], f32, tag="op")
                    for j in range(4):
                        for m in range(2):
                            nc.tensor.matmul(
                                out_ps[:, j, :],
                                y_sc[:, m, 128 * j:128 * (j + 1)],
                                m_sb[m][:],
                                start=(m == 0), stop=(m == 1))
                    if (not copy_act_all) and u % 3 == 2:
                        nc.vector.tensor_copy(out_sb[:].opt(), out_ps[:].opt())
                    else:
                        nc.scalar.activation(out_sb[:], out_ps[:], AF.Copy)
                    nc.sync.dma_start(out_r[u], out_sb[:])

                next_mid = 0
                next_back = 0
                for u in range(PREFILL, NP):
                    issue_front(u)
                    issued = 0
                    while next_mid <= u - 2 and issued < 2:
                        issue_mid(next_mid)
                        next_mid += 1
                        issued += 1
                    if next_back < next_mid - 3:
                        issue_back(next_back)
                        next_back += 1
                while next_mid < NP:
                    issue_mid(next_mid)
                    next_mid += 1
                    if next_back < next_mid - 3:
                        issue_back(next_back)
                        next_back += 1
                while next_back < NP:
                    issue_back(next_back)
                    next_back += 1

    if split_waits:
        _split_multiwaits(nc)
    return nc


def _split_multiwaits(nc, limit=1):
    """This container's walrus rejects instructions carrying more than a
    couple of sync waits (CoreV3 setupSyncWait: 'Too many sync wait
    commands'). Splitting extra waits onto preceding same-engine NoOps is
    semantically identical on an in-order engine."""
    from concourse import mybir

    f = nc.m.functions[0]
    for b in f.blocks:
        new_insts = []
        for inst in b.instructions:
            si = getattr(inst, "sync_info", None)
            waits = list(si.on_wait) if (si and si.on_wait) else []
            if len(waits) > limit:
                head, keep = waits[:-limit], waits[-limit:]
                for w0 in range(0, len(head), limit):
                    nop = mybir.InstNoOp(
                        name=nc.get_next_instruction_name(), ins=[], outs=[])
                    nop.engine = inst.engine
                    nop.sync_info = mybir.SyncInfo(
                        on_wait=head[w0:w0 + limit], on_update=[])
                    new_insts.append(nop)
                inst.sync_info = mybir.SyncInfo(
                    on_wait=keep, on_update=list(si.on_update or []))
            new_insts.append(inst)
        b.instructions[:] = new_insts


def _build_null_nc(lh=LH):
    """Minimal program with the same I/O signature (for dispatch-overhead
    measurement in test.py)."""
    import concourse.bass as bass
    import concourse.mybir as mybir
    import concourse.tile as tile

    f32 = mybir.dt.float32
    bf16 = mybir.dt.bfloat16
    nc = bass.Bass("TRN2", target_bir_lowering=False, debug=False,
                   num_devices=N_CORES)
    xT = nc.dram_tensor("xT", [C, lh], bf16, kind="ExternalInput")
    nc.dram_tensor("xn", [lh, 258], bf16, kind="ExternalInput")
    nc.dram_tensor("w_qkv", [C, 3 * C], bf16, kind="ExternalInput")
    nc.dram_tensor("w_proj", [C, C], bf16, kind="ExternalInput")
    nc.dram_tensor("eyeid", [128, 256], f32, kind="ExternalInput")
    out = nc.dram_tensor("out", [lh, C], f32, kind="ExternalOutput")
    with tile.TileContext(nc) as tc:
        with tc.tile_pool(name="p", bufs=1) as p:
            t = p.tile([1, 512], bf16, tag="t", name="t")
            nc.sync.dma_start(t[:], xT[0:1, 0:512])
            nc.sync.dma_start(out[0:1, :], t[:].bitcast(f32))
    _split_multiwaits(nc)
    return nc


class _Runner:
    """Cached jit(shard_map(bass_exec)) over the 8 axon trn2 cores."""

    def __init__(self, nc):
        import jax
        import jax.numpy as jnp
        from jax.sharding import Mesh, PartitionSpec
        from jax.experimental.shard_map import shard_map
        import concourse.mybir as mybir
        from concourse import bass2jax

        bass2jax.install_neuronx_cc_hook()
        self.jax, self.jnp = jax, jnp
        self.nc = nc

        partition_name = (nc.partition_id_tensor.name
                          if nc.partition_id_tensor else None)
        in_names, out_names, out_avals = [], [], []
        for alloc in nc.m.functions[0].allocations:
            if not isinstance(alloc, mybir.MemoryLocationSet):
                continue
            name = alloc.memorylocations[0].name
            if alloc.kind == "ExternalInput":
                if name != partition_name:
                    in_names.append(name)
            elif alloc.kind == "ExternalOutput":
                out_names.append(name)
                out_avals.append(jax.core.ShapedArray(
                    tuple(alloc.tensor_shape), mybir.dt.np(alloc.dtype)))
        assert nc.dbg_addr is None
        self.in_names, self.out_names, self.out_avals = in_names, out_names, out_avals
        n_params = len(in_names)
        all_in_names = in_names + out_names
        if partition_name is not None:
            all_in_names = all_in_names + [partition_name]
        all_in_names = tuple(all_in_names)
        self.all_in_names = all_in_names
        self.partition_name = partition_name

        def _body(*args):
            operands = list(args)
            if partition_name is not None:
                operands.append(bass2jax.partition_id_tensor())
            outs = bass2jax._bass_exec_p.bind(
                *operands,
                out_avals=tuple(out_avals),
                in_names=all_in_names,
                out_names=tuple(out_names),
                lowering_input_output_aliases=(),
                sim_require_finite=True,
                sim_require_nnan=True,
                nc=nc,
            )
            return tuple(outs)

        devices = jax.devices()[:N_CORES]
        self.mesh = Mesh(np.asarray(devices), ("core",))
        spec = PartitionSpec("core")
        n_outs = len(out_names)
        self.donate = tuple(range(n_params, n_params + n_outs))
        self.fn = jax.jit(
            shard_map(_body, mesh=self.mesh, in_specs=(spec,) * (n_params + n_outs),
                      out_specs=(spec,) * n_outs, check_rep=False),
            donate_argnums=self.donate, keep_unused=True)
        self.sharding = jax.sharding.NamedSharding(self.mesh, spec)

        def _zeros():
            return tuple(
                jnp.zeros((N_CORES * a.shape[0], *a.shape[1:]), a.dtype)
                for a in out_avals)
        self.zeros_fn = jax.jit(_zeros, out_shardings=(self.sharding,) * n_outs)

    def place_inputs(self, in_maps):
        concat = [np.concatenate([np.asarray(m[n]) for m in in_maps], axis=0)
                  for n in self.in_names]
        return [self.jax.device_put(a, self.sharding) for a in concat]

    def call(self, dev_in):
        outs = self.fn(*dev_in, *self.zeros_fn())
        self.jax.block_until_ready(outs)
        return outs

    def run(self, in_maps):
        outs = self.call(self.place_inputs(in_maps))
        res = []
        for c in range(N_CORES):
            res.append({n: np.asarray(outs[i]).reshape(
                N_CORES, *self.out_avals[i].shape)[c]
                for i, n in enumerate(self.out_names)})
        return res


def _get_runner(lh=LH, null=False):
    key = (lh, null)
    if key not in _NC_CACHE:
        nc = _build_null_nc(lh) if null else _build_nc(lh)
        _NC_CACHE[key] = _Runner(nc)
    return _NC_CACHE[key]


def _make_eyeid():
    eye = np.kron(np.eye(4, dtype=np.float32), np.ones((32, 32), np.float32))
    return np.concatenate([eye, np.eye(128, dtype=np.float32)], axis=1)


def _make_in_maps(x, W_qkv, W_proj, lh=LH, xt_f32r=False):
    import ml_dtypes

    bf16 = ml_dtypes.bfloat16
    ncores_b = B * (L // lh)
    xb = np.ascontiguousarray(x.reshape(B, L // lh, lh, C))
    eyeid = _make_eyeid()
    xdt = np.float32 if xt_f32r else bf16
    w = np.ascontiguousarray(W_qkv).astype(xdt)
    wp = np.ascontiguousarray(W_proj).astype(bf16)
    in_maps = []
    for c in range(ncores_b):
        bb, hh = divmod(c, L // lh)
        xh = xb[bb, hh]                                   # [lh, C] f32
        xTc = np.ascontiguousarray(xh.T).astype(xdt)      # [C, lh]
        xnc = np.empty((lh, 258), dtype=bf16)
        xnc[:, 0:C] = xh.astype(bf16)
        xnc[:, C:258] = bf16(1.0)
        in_maps.append({"xT": xTc, "xn": xnc, "w_qkv": w, "w_proj": wp,
                        "eyeid": eyeid})
    return in_maps


def _assemble(results):
    outs = [results[c]["out"] for c in range(N_CORES)]
    y = np.stack(outs).reshape(B, 2, LH, C).reshape(B, L, C)
    return np.ascontiguousarray(y.reshape(BV, HW, C), dtype=np.float32)


def _run(x, W_qkv, W_proj, b_proj):
    runner = _get_runner(LH)
    in_maps = _make_in_maps(x, W_qkv, W_proj)
    res = _assemble(runner.run(in_maps))
    if np.any(b_proj):
        res = res + np.asarray(b_proj, np.float32)
    return res


def kernel(x, W_qkv, W_proj, b_proj):
    return _run(np.asarray(x, np.float32), np.asarray(W_qkv, np.float32),
                np.asarray(W_proj, np.float32), np.asarray(b_proj, np.float32))
